# revision 18
# baseline (speedup 1.0000x reference)
"""Trainium2 Bass kernel for nn_Attention (B=4, S=1024, DIM=1024, H=16, Dh=64).

Sharding: 8 cores = 4 batches x 2 head-groups (8 heads / 512 inner channels
each).  Each core computes q/k/v projections for its head shard, RoPE,
attention, and a partial (transposed) output projection; the host sums the
two head-group partials per batch, transposes, zeroes masked rows, and
concatenates batches.

v4 dataflow (fp16, latency-shaped):
  Per-kt chunked DMAs; K0/Q0 projections run kt-outer chasing the DMA
  stream, with ~4us of junk matmuls folded into the first DMA wait so the
  PE HAM clock un-throttles before the real work.  RoPE as before.
  V is zippered with the (mt=0, c2=0) scores+exp phase so ACT starts its
  73us exp marathon as early as possible; after that the attention is
  (mt, c2)-outer: 8x [scores pair (fp16) -> exp -> PV], with PV using V
  augmented by a ones column (M=65) so the rowsum lands in PSUM partition
  64 -- no separate rowsum matmuls.  Next row-tile's projections are
  zippered 4-or-2 per kt.  Normalization: reciprocal on DVE, DRAM-bounce
  partition-broadcast + multiplies on GpSimd (so the DVE FIFO never blocks
  on DMA latency); the last row-tile instead uses a PE-broadcast matmul.
  Output projection is flipped (out^T = Wo^T @ attn^T): the mt=0/1 partial
  runs as PE filler inside the mt=3 phases, the mt=2/3 remainder + bias +
  partial-add forms the tail; host transposes and masks.
"""

import numpy as np

B, S, DIM, HEADS, HEAD_DIM = 4, 1024, 1024, 16, 64
INNER = HEADS * HEAD_DIM
HG = 2                      # head groups (tensor-parallel shards)
DSH = INNER // HG           # 512 inner channels per core
HSH = HEADS // HG           # 8 heads per core
NCORES = B * HG
KT = DIM // 128             # 8 contraction tiles
MT = DSH // 128             # 4 row tiles for Q^T/K^T
ST = S // 128               # 8 seq tiles
VW = 65                     # per-head V width (64 V + 1 ones)
MASK_NEG = -80.0

_CACHE = {}


def _build():
    import concourse.tile as tile
    from concourse import bacc, mybir

    f32 = mybir.dt.float32
    f16 = mybir.dt.float16
    AF = mybir.ActivationFunctionType
    OP = mybir.AluOpType

    nc = bacc.Bacc("TRN2", target_bir_lowering=False, debug=False)

    xT_d = nc.dram_tensor("xT", [128, KT, S], f16, kind="ExternalInput")
    wq_d = nc.dram_tensor("wq", [128, KT, MT, 128], f16, kind="ExternalInput")
    wk_d = nc.dram_tensor("wk", [128, KT, MT, 128], f16, kind="ExternalInput")
    wv_d = nc.dram_tensor("wv", [128, KT, DSH], f16, kind="ExternalInput")
    wo_d = nc.dram_tensor("wo", [128, MT, DIM], f16, kind="ExternalInput")
    bq_d = nc.dram_tensor("bq", [128, MT], f32, kind="ExternalInput")
    bk_d = nc.dram_tensor("bk", [128, MT], f32, kind="ExternalInput")
    bv_d = nc.dram_tensor("bv", [1, DSH], f16, kind="ExternalInput")
    boT_d = nc.dram_tensor("boT", [128, ST], f32, kind="ExternalInput")
    cos_d = nc.dram_tensor("cos2", [128, S], f32, kind="ExternalInput")
    sin_d = nc.dram_tensor("sin2", [128, S], f32, kind="ExternalInput")
    prt_d = nc.dram_tensor("prt", [128, 128], f16, kind="ExternalInput")
    maskb_d = nc.dram_tensor("maskb", [128, ST], f32, kind="ExternalInput")
    out_d = nc.dram_tensor("out", [DIM, S], f32, kind="ExternalOutput")

    with tile.TileContext(nc) as tc, \
         tc.tile_pool(name="persist", bufs=1) as persist:
        # ---- persistent tiles --------------------------------------------
        xT = persist.tile([128, KT, S], f16)
        wq = persist.tile([128, KT, MT, 128], f16)
        wk = persist.tile([128, KT, MT, 128], f16)
        wv = persist.tile([128, KT, DSH], f16)
        wo = persist.tile([128, MT, DIM], f16)
        bq = persist.tile([128, MT], f32)
        bk = persist.tile([128, MT], f32)
        bv = persist.tile([1, DSH], f16)
        boT = persist.tile([128, ST], f32)
        maskb = persist.tile([128, ST], f32)
        qT = persist.tile([128, MT, S], f16)
        kT = persist.tile([128, MT, S], f16)
        vv = persist.tile([128, ST, HSH, VW], f16)
        attU = persist.tile([128, MT, S], f16)     # unnormalized attn^T
        attN = persist.tile([128, MT, S], f16)     # normalized
        oprt = persist.tile([128, ST, S], f32)     # phase-3 partials (mt 0-1)
        ones = persist.tile([1, 128], f16)
        ones4 = persist.tile([97, HEAD_DIM], f16)
        # rowsums at partitions 0/32/64/96 (col-group constraint)
        rssum = persist.tile([97, MT, 512], f32)
        recq = persist.tile([97, MT, 512], f16)
        recf = persist.tile([97, 512], f32)
        recd = nc.dram_tensor("recd", [97, MT, 512], f16)

        # ---- chunked input DMAs (per-kt so compute can chase the stream) -
        for kt in range(KT):
            nc.sync.dma_start(out=xT[:, kt], in_=xT_d.ap()[:, kt])
            nc.scalar.dma_start(out=wk[:, kt], in_=wk_d.ap()[:, kt])
            nc.gpsimd.dma_start(out=wq[:, kt], in_=wq_d.ap()[:, kt])
        for t, d in [(bq, bq_d), (bk, bk_d), (bv, bv_d), (maskb, maskb_d)]:
            nc.gpsimd.dma_start(out=t[:], in_=d.ap())
        for kt in range(KT):
            nc.scalar.dma_start(out=wv[:, kt], in_=wv_d.ap()[:, kt])
        nc.scalar.dma_start(out=wo[:], in_=wo_d.ap())
        nc.gpsimd.dma_start(out=boT[:], in_=boT_d.ap())

        with tc.tile_pool(name="w1", bufs=1) as w1:
            cos2 = w1.tile([128, S], f32)
            sin2 = w1.tile([128, S], f32)
            prt = w1.tile([128, 128], f16)
            for t, d in [(cos2, cos_d), (sin2, sin_d), (prt, prt_d)]:
                nc.gpsimd.dma_start(out=t[:], in_=d.ap())
            ones_f = w1.tile([128, 128], f32)
            nc.vector.memset(ones_f[:], 1.0)
            nc.vector.tensor_copy(ones[:], ones_f[0:1, :])
            nc.vector.tensor_copy(ones4[:], ones_f[0:97, 0:HEAD_DIM])
            # ones column of V_aug (per head, col 64)
            nc.vector.tensor_copy(
                vv[:, :, :, 64:65].rearrange("p a b c -> p (a b c)"),
                ones_f[:, 0:ST * HSH])
            # only 4 rows per mt are real rowsums; fill the rest so the
            # batched reciprocal never sees uninitialized SBUF
            nc.vector.memset(rssum[:], 1.0)

            def rope_apply(dst, b, c2, ps, pppool, sbpool):
                # row-tile 0 only: RoPE on the first 64 flat channels
                # (rows 64-127 and the hg=1 core get identity via
                # cos=1/sin=0 from the host).
                sl = slice(c2 * 512, (c2 + 1) * 512)
                sinp = sbpool.tile([128, 512], f16, tag="sinp", name="sinp")
                nc.vector.scalar_tensor_tensor(
                    sinp[:], ps[:], b[:, 0:1],
                    sin2[:, sl], op0=OP.add, op1=OP.mult)
                cosp = sbpool.tile([128, 512], f32, tag="cosp", name="cosp")
                nc.vector.scalar_tensor_tensor(
                    cosp[:], ps[:], b[:, 0:1],
                    cos2[:, sl], op0=OP.add, op1=OP.mult)
                pp = pppool.tile([128, 512], f32, tag="pp", name="pp")
                nc.tensor.matmul(out=pp[:], lhsT=prt[:], rhs=sinp[:],
                                 start=True, stop=True)
                nc.vector.tensor_tensor(
                    dst[:, 0, sl], cosp[:], pp[:], op=OP.add)

            # ---- prologue: K0/Q0 kt-outer (chases the DMA stream) -------
            with tc.tile_pool(name="kqps", bufs=1, space="PSUM") as kqps, \
                 tc.tile_pool(name="p1pp", bufs=2, space="PSUM") as p1pp, \
                 tc.tile_pool(name="p1sb", bufs=3) as p1sb:
                kq = {}
                for di, (dst, w, b) in enumerate(
                        ((kT, wk, bk), (qT, wq, bq))):
                    for c2 in range(2):
                        kq[di, c2] = kqps.tile([128, 512], f32,
                                               tag=f"kq{di}{c2}",
                                               name=f"kq{di}{c2}")
                warm = kqps.tile([128, 512], f32, tag="warm", name="warm")
                for kt in range(KT):
                    for di, (dst, w, b) in enumerate(
                            ((kT, wk, bk), (qT, wq, bq))):
                        for c2 in range(2):
                            sl = slice(c2 * 512, (c2 + 1) * 512)
                            nc.tensor.matmul(
                                out=kq[di, c2][:],
                                lhsT=w[:, kt, 0, :],
                                rhs=xT[:, kt, sl],
                                start=(kt == 0), stop=(kt == KT - 1))
                    if kt == 1:
                        # HAM warm-up: ~4us of back-to-back junk matmuls on
                        # already-landed chunks fills the DMA wait and
                        # un-throttles the PE clock before the real work.
                        for _ in range(10):
                            nc.tensor.matmul(
                                out=warm[:], lhsT=wk[:, 0, 0, :],
                                rhs=xT[:, 0, 0:512], start=True, stop=True)
                # consume the warm tile so nothing can dead-code it away
                nc.vector.tensor_copy(ones_f[0:1, 100:116], warm[0:1, 0:16])
                for di, c2 in ((0, 0), (1, 0), (0, 1), (1, 1)):
                    dst, w, b = ((kT, wk, bk), (qT, wq, bq))[di]
                    rope_apply(dst, b, c2, kq[di, c2], p1pp, p1sb)

            # ---- phase 2 pools ------------------------------------------
            with tc.tile_pool(name="sch", bufs=2, space="PSUM") as schp, \
                 tc.tile_pool(name="ptp", bufs=8) as ptp, \
                 tc.tile_pool(name="p2sb", bufs=2) as p2sb:

                def emit_scores(mt, c2, kt):
                    qsl = slice(c2 * 512, (c2 + 1) * 512)
                    sc = schp.tile([128, S], f32, tag="sch", name="sch")
                    for hh in range(2):
                        ph = hh * 64
                        nc.tensor.matmul(
                            out=sc[:, hh * 512:hh * 512 + 512],
                            lhsT=kT[ph:ph + 64, mt, kt * 128:(kt + 1) * 128],
                            rhs=qT[ph:ph + 64, mt, qsl],
                            start=True, stop=True, tile_position=(ph, 0))
                    return sc

                def emit_exp(pt, sc, kt):
                    nc.scalar.activation(
                        pt[:], sc[:], AF.Exp,
                        bias=maskb[:, kt:kt + 1], scale=0.125)

                def proj_v(st, pool):
                    ps = pool.tile([128, DSH], f32, tag="vps", name="vps")
                    nc.tensor.matmul(out=ps[:], lhsT=ones[0:1, :],
                                     rhs=bv[:], start=True, stop=False)
                    for kt in range(KT):
                        nc.tensor.matmul(
                            out=ps[:],
                            lhsT=xT[:, kt, st * 128:(st + 1) * 128],
                            rhs=wv[:, kt, :],
                            start=False, stop=(kt == KT - 1))
                    nc.vector.tensor_copy(
                        vv[:, st, :, 0:64],
                        ps[:].rearrange("p (h d) -> p h d", h=HSH))

                # V zippered with (mt=0, c2=0) scores+exps: ACT starts its
                # exp marathon while PE streams V.
                pts = {}          # (mt, c2, kt) -> pt tile
                with tc.tile_pool(name="vps", bufs=2, space="PSUM") as vps:
                    for st in range(ST):
                        proj_v(st, vps)
                        sc = emit_scores(0, 0, st)
                        pt = ptp.tile([128, S], f16, tag="pt", name="pt")
                        pts[0, 0, st] = pt
                        emit_exp(pt, sc, st)

                def proj_gen(mt):
                    # one matmul per next(); bias epilogue on the last.
                    for dst, w, b in ((kT, wk, bk), (qT, wq, bq)):
                        for c2 in range(2):
                            sl = slice(c2 * 512, (c2 + 1) * 512)
                            ps = p1ps.tile([128, 512], f32, tag="ps",
                                           name="ps")
                            for kt in range(KT):
                                nc.tensor.matmul(
                                    out=ps[:], lhsT=w[:, kt, mt, :],
                                    rhs=xT[:, kt, sl],
                                    start=(kt == 0), stop=(kt == KT - 1))
                                if kt == KT - 1:
                                    nc.vector.tensor_scalar(
                                        dst[:, mt, sl], ps[:],
                                        b[:, mt:mt + 1], None, op0=OP.add)
                                yield
                    while True:
                        yield

                def emit_pv(at, mt, c2, kt):
                    pt = pts.pop((mt, c2, kt))
                    for hh in range(2):
                        nc.tensor.matmul(
                            out=at[hh][:],
                            lhsT=vv[:, kt, mt * 2 + hh, :],
                            rhs=pt[:, hh * 512:hh * 512 + 512],
                            start=(kt == 0), stop=(kt == KT - 1))

                def epilogue(at, mt, c2):
                    qsl = slice(c2 * 512, (c2 + 1) * 512)
                    for hh in range(2):
                        r = 32 * (hh * 2 + c2)
                        nc.vector.tensor_copy(
                            rssum[r:r + 1, mt, :], at[hh][64:65, :])
                        nc.vector.tensor_copy(
                            attU[hh * 64:hh * 64 + 64, mt, qsl],
                            at[hh][0:64, :])

                def normalize(mt):
                    # reciprocal of the 4 rowsum rows on DVE; the DRAM-
                    # bounce partition-broadcast and the scaling multiplies
                    # run on GpSimd so the DVE FIFO never waits on DMA.
                    nc.vector.reciprocal_approx_fast(
                        recf[:], rssum[:, mt, :])
                    nc.vector.tensor_copy(recq[:, mt, :], recf[:])
                    nc.gpsimd.dma_start(out=recd.ap()[:, mt, :],
                                        in_=recq[:, mt, :])
                    for hh in range(2):
                        ph = hh * 64
                        rb = p2sb.tile([128, 2, 512], f32, tag="rb",
                                       name="rb")
                        nc.gpsimd.dma_start(
                            out=rb[ph:ph + 64],
                            in_=recd.ap()[64 * hh:64 * hh + 33:32,
                                          mt, :].partition_broadcast(HEAD_DIM))
                        nc.gpsimd.tensor_tensor(
                            attN[ph:ph + 64, mt, :], attU[ph:ph + 64, mt, :],
                            rb[ph:ph + 64].rearrange("p a b -> p (a b)"),
                            op=OP.mult)

                with tc.tile_pool(name="atp", bufs=1, space="PSUM") as atp, \
                     tc.tile_pool(name="p1ps", bufs=2, space="PSUM") as p1ps:
                    def p3partial_gen():
                        # phase-3 partials (mt 0-1 only -- normalized long
                        # ago, so these can never stall the in-order PE
                        # queue): filler for the last row-tile's phases.
                        for dt in range(ST):
                            dsl = slice(dt * 128, (dt + 1) * 128)
                            for c2 in range(2):
                                qsl = slice(c2 * 512, (c2 + 1) * 512)
                                ps = p1ps.tile([128, 512], f32, tag="ps",
                                               name="ps")
                                for mt in range(2):
                                    nc.tensor.matmul(
                                        out=ps[:], lhsT=wo[:, mt, dsl],
                                        rhs=attN[:, mt, qsl],
                                        start=(mt == 0), stop=(mt == 1))
                                    yield
                                nc.vector.tensor_copy(oprt[:, dt, qsl],
                                                      ps[:])
                        while True:
                            yield

                    gen = proj_gen(1)
                    p3gen = p3partial_gen()
                    for mt in range(MT):
                        for c2 in range(2):
                            if c2 == 0 and mt > 0:
                                normalize(mt - 1)
                            at = {hh: atp.tile([65, 512], f32,
                                               tag=f"at{hh}",
                                               name=f"at{hh}")
                                  for hh in range(2)}
                            for kt in range(ST):
                                if not (mt == 0 and c2 == 0):
                                    sc = emit_scores(mt, c2, kt)
                                    pts[mt, c2, kt] = ptp.tile(
                                        [128, S], f16, tag="pt", name="pt")
                                    emit_exp(pts[mt, c2, kt], sc, kt)
                                if mt == 0:
                                    np1, np2 = (1, kt % 2) if c2 == 0 \
                                        else (2, (kt + 1) % 2)
                                else:
                                    np1, np2 = 1, 1
                                for _ in range(np1):
                                    next(gen)
                                emit_pv(at, mt, c2, kt)
                                for _ in range(np2):
                                    next(gen)
                                if mt == 3:
                                    for _ in range(2):
                                        next(p3gen)
                            epilogue(at, mt, c2)
                            if c2 == 1:
                                gen = proj_gen(mt + 2) if mt + 2 <= MT - 1 \
                                    else iter(lambda: None, 0)
                    # last row-tile: reciprocal chain, then PE-broadcast
                    # normalize per (c2, hh) -- no DMA bounce on the
                    # critical path -- interleaved with the output tail.
                    nc.vector.reciprocal_approx_fast(recf[:],
                                                     rssum[:, 3, :])
                    nc.vector.tensor_copy(recq[:, 3, :], recf[:])
                    for _ in range(2):   # flush the generator's last spill
                        next(p3gen)
                    for c2 in range(2):
                        qsl = slice(c2 * 512, (c2 + 1) * 512)
                        for hh in range(2):
                            ph = hh * 64
                            r = 32 * (hh * 2 + c2)
                            rbp = p1ps.tile([128, 512], f32, tag="ps",
                                            name="rbp")
                            nc.tensor.matmul(
                                out=rbp[0:64, :], lhsT=ones4[r:r + 1, :],
                                rhs=recq[r:r + 1, 3, :],
                                start=True, stop=True, tile_position=(r, 0))
                            nc.vector.tensor_tensor(
                                attN[ph:ph + 64, 3, qsl],
                                attU[ph:ph + 64, 3, qsl],
                                rbp[0:64, :], op=OP.mult)
                        # tail: mt=2 + mt=3 on top of the partial, plus
                        # bias; DMA out as computed.
                        for dt in range(ST):
                            dsl = slice(dt * 128, (dt + 1) * 128)
                            ps = p1ps.tile([128, 512], f32, tag="ps",
                                           name="ps")
                            nc.tensor.matmul(
                                out=ps[:], lhsT=wo[:, 2, dsl],
                                rhs=attN[:, 2, qsl], start=True, stop=False)
                            nc.tensor.matmul(
                                out=ps[:], lhsT=wo[:, 3, dsl],
                                rhs=attN[:, 3, qsl], start=False, stop=True)
                            ob = p2sb.tile([128, 512], f32, tag="ob",
                                           name="ob")
                            if dt % 2 == 0:
                                nc.vector.scalar_tensor_tensor(
                                    ob[:], ps[:], boT[:, dt:dt + 1],
                                    oprt[:, dt, qsl], op0=OP.add, op1=OP.add)
                            else:
                                # ACT is idle post-exps and can read PSUM;
                                # GpSimd finishes the SBUF-only add.
                                obA = p2sb.tile([128, 512], f32, tag="obA",
                                                name="obA")
                                nc.scalar.activation(
                                    obA[:], ps[:], AF.Identity,
                                    bias=boT[:, dt:dt + 1])
                                nc.gpsimd.tensor_tensor(
                                    ob[:], obA[:], oprt[:, dt, qsl],
                                    op=OP.add)
                            deng = nc.sync if dt % 2 == 0 else nc.scalar
                            deng.dma_start(out=out_d.ap()[dsl, qsl],
                                           in_=ob[:])

    nc.compile()
    return nc


def _get_nc():
    if "nc" not in _CACHE:
        _CACHE["nc"] = _build()
    return _CACHE["nc"]


def _prep_inputs(x, mask, freqs, Wq, bq, Wk, bk, Wv, bv, Wo, bo):
    f = np.asarray(freqs, np.float32)[0]              # [S, HEAD_DIM]
    # reference rotates only the first rot_dim=64 channels of the FLAT
    # inner dim -> rows 0-63 of row-tile 0 on the hg=0 core; everything
    # else is identity (cos=1, sin=0).
    cos2 = np.ones((128, S), np.float32)
    sin2 = np.zeros((128, S), np.float32)
    cos2[0:HEAD_DIM] = np.cos(f.T)
    sin2[0:HEAD_DIM] = np.sin(f.T)
    ident = np.ones((128, S), np.float32)
    identz = np.zeros((128, S), np.float32)

    prt = np.zeros((128, 128), np.float16)            # P_rot^T
    i = np.arange(0, 128, 2)
    prt[i + 1, i] = -1.0                              # P_rot[2i, 2i+1] = -1
    prt[i, i + 1] = 1.0                               # P_rot[2i+1, 2i] = +1

    def lhsT_w(w):                                    # [DIM, DSH] -> lhsT tiles
        return np.ascontiguousarray(
            w.reshape(KT, 128, MT, 128).transpose(1, 0, 2, 3)
        ).astype(np.float16)

    def col(b):                                       # [DSH] -> [128, MT]
        return np.ascontiguousarray(b.reshape(MT, 128).T.astype(np.float32))

    in_maps = []
    for b in range(B):
        xT = np.ascontiguousarray(
            np.asarray(x[b], np.float32).T.reshape(KT, 128, S)
            .transpose(1, 0, 2)).astype(np.float16)
        m = np.asarray(mask[b])
        maskb = np.ascontiguousarray(
            np.where(m, 0.0, MASK_NEG).astype(np.float32).reshape(ST, 128).T)
        for hg in range(HG):
            dsl = slice(hg * DSH, (hg + 1) * DSH)
            in_maps.append({
                "xT": xT,
                "wq": lhsT_w(np.asarray(Wq, np.float32)[:, dsl]),
                "wk": lhsT_w(np.asarray(Wk, np.float32)[:, dsl]),
                "wv": np.ascontiguousarray(
                    np.asarray(Wv, np.float32)[:, dsl]
                    .reshape(KT, 128, DSH).transpose(1, 0, 2))
                    .astype(np.float16),
                "wo": np.ascontiguousarray(
                    np.asarray(Wo, np.float32)[dsl, :]
                    .reshape(MT, 128, DIM).transpose(1, 0, 2))
                    .astype(np.float16),
                "bq": col(np.asarray(bq, np.float32)[dsl]),
                "bk": col(np.asarray(bk, np.float32)[dsl]),
                "bv": np.asarray(bv, np.float32)[None, dsl]
                    .astype(np.float16).copy(),
                "boT": np.ascontiguousarray(
                    (np.asarray(bo, np.float32) * 0.5).reshape(ST, 128).T),
                "cos2": cos2 if hg == 0 else ident,
                "sin2": sin2 if hg == 0 else identz,
                "prt": prt,
                "maskb": maskb,
            })
    return in_maps


def run(trace=False, **inputs):
    from concourse import bass_utils
    if trace:
        _install_ntff_hook()
    nc = _get_nc()
    in_maps = _prep_inputs(**inputs)
    res = bass_utils.run_bass_kernel_spmd(
        nc, in_maps, core_ids=list(range(NCORES)), trace=trace)
    mask = np.asarray(inputs["mask"])
    out = np.empty((B, S, DIM), np.float32)
    for b in range(B):
        ot = res.results[2 * b]["out"] + res.results[2 * b + 1]["out"]
        out[b] = ot.T
        out[b][~mask[b]] = 0.0
    return out, res


def kernel(**inputs):
    out, _ = run(trace=False, **inputs)
    return out


def _install_ntff_hook():
    """Register the axon NTFF profiling hook missing from the antenv stub."""
    import sys, types
    try:
        import antenv.axon_hooks  # noqa: F401
        return
    except ImportError:
        pass
    from trn_agent_boot.trn_boot import _ntff_profile_via_ctypes
    hook = _ntff_profile_via_ctypes('/opt/axon/libaxon_pjrt.so')
    mod = types.ModuleType('antenv.axon_hooks')
    mod.get_axon_ntff_profile_hook = lambda: hook
    mod.set_axon_ntff_profile_hook = lambda h: None
    sys.modules['antenv.axon_hooks'] = mod


# revision 21
# speedup vs baseline: 1.1221x; 1.1221x over previous
"""Trainium2 Bass kernel for nn_Attention (B=4, S=1024, DIM=1024, H=16, Dh=64).

Sharding: 8 cores = 4 batches x 2 head-groups (8 heads / 512 inner channels
each).  Each core computes q/k/v projections for its head shard, RoPE,
attention, and a partial (transposed) output projection; the host sums the
two head-group partials per batch, transposes, zeroes masked rows, and
concatenates batches.

v4 dataflow (fp16, latency-shaped):
  Per-kt chunked DMAs; K0/Q0 projections run kt-outer chasing the DMA
  stream, with ~4us of junk matmuls folded into the first DMA wait so the
  PE HAM clock un-throttles before the real work.  RoPE as before.
  V is zippered with the (mt=0, c2=0) scores+exp phase so ACT starts its
  73us exp marathon as early as possible; after that the attention is
  (mt, c2)-outer: 8x [scores pair (fp16) -> exp -> PV], with PV using V
  augmented by a ones column (M=65) so the rowsum lands in PSUM partition
  64 -- no separate rowsum matmuls.  Next row-tile's projections are
  zippered 4-or-2 per kt.  Normalization: reciprocal on DVE, DRAM-bounce
  partition-broadcast + multiplies on GpSimd (so the DVE FIFO never blocks
  on DMA latency); the last row-tile instead uses a PE-broadcast matmul.
  Output projection is flipped (out^T = Wo^T @ attn^T): the mt=0/1 partial
  runs as PE filler inside the mt=3 phases, the mt=2/3 remainder + bias +
  partial-add forms the tail; host transposes and masks.
"""

import numpy as np

B, S, DIM, HEADS, HEAD_DIM = 4, 1024, 1024, 16, 64
INNER = HEADS * HEAD_DIM
HG = 2                      # head groups (tensor-parallel shards)
DSH = INNER // HG           # 512 inner channels per core
HSH = HEADS // HG           # 8 heads per core
NCORES = B * HG
KT = DIM // 128             # 8 contraction tiles
MT = DSH // 128             # 4 row tiles for Q^T/K^T
ST = S // 128               # 8 seq tiles
VW = 65                     # per-head V width (64 V + 1 ones)
MASK_NEG = -80.0

_CACHE = {}


def _build():
    import concourse.tile as tile
    from concourse import bacc, mybir

    f32 = mybir.dt.float32
    f16 = mybir.dt.float16
    AF = mybir.ActivationFunctionType
    OP = mybir.AluOpType

    nc = bacc.Bacc("TRN2", target_bir_lowering=False, debug=False)

    xT_d = nc.dram_tensor("xT", [128, KT, S], f16, kind="ExternalInput")
    wq_d = nc.dram_tensor("wq", [128, KT, MT, 128], f16, kind="ExternalInput")
    wk_d = nc.dram_tensor("wk", [128, KT, MT, 128], f16, kind="ExternalInput")
    wv_d = nc.dram_tensor("wv", [128, KT, DSH], f16, kind="ExternalInput")
    wo_d = nc.dram_tensor("wo", [128, MT, DIM], f16, kind="ExternalInput")
    bq_d = nc.dram_tensor("bq", [128, MT], f32, kind="ExternalInput")
    bk_d = nc.dram_tensor("bk", [128, MT], f32, kind="ExternalInput")
    bv_d = nc.dram_tensor("bv", [1, DSH], f16, kind="ExternalInput")
    boT_d = nc.dram_tensor("boT", [128, ST], f32, kind="ExternalInput")
    cos_d = nc.dram_tensor("cos2", [128, S], f32, kind="ExternalInput")
    sin_d = nc.dram_tensor("sin2", [128, S], f32, kind="ExternalInput")
    prt_d = nc.dram_tensor("prt", [128, 128], f16, kind="ExternalInput")
    maskb_d = nc.dram_tensor("maskb", [128, ST], f32, kind="ExternalInput")
    out_d = nc.dram_tensor("out", [DIM, S], f32, kind="ExternalOutput")

    with tile.TileContext(nc) as tc, \
         tc.tile_pool(name="persist", bufs=1) as persist:
        # ---- persistent tiles --------------------------------------------
        xT = persist.tile([128, KT, S], f16)
        wq = persist.tile([128, KT, MT, 128], f16)
        wk = persist.tile([128, KT, MT, 128], f16)
        wv = persist.tile([128, KT, DSH], f16)
        wo = persist.tile([128, MT, DIM], f16)
        bq = persist.tile([128, MT], f32)
        bk = persist.tile([128, MT], f32)
        bv = persist.tile([1, DSH], f16)
        boT = persist.tile([128, ST], f32)
        maskb = persist.tile([128, ST], f32)
        qT = persist.tile([128, MT, S], f16)
        kT = persist.tile([128, MT, S], f16)
        vv = persist.tile([128, ST, HSH, VW], f16)
        attU = persist.tile([128, MT, S], f16)     # unnormalized attn^T
        attN = persist.tile([128, MT, S], f16)     # normalized
        oprt = persist.tile([128, ST, S], f32)     # phase-3 partials (mt 0-1)
        ones = persist.tile([1, 128], f16)
        ones4 = persist.tile([97, HEAD_DIM], f16)
        # rowsums at partitions 0/32/64/96 (col-group constraint)
        rssum = persist.tile([97, MT, 512], f32)
        recq = persist.tile([97, MT, 512], f16)
        recf = persist.tile([97, 512], f32)
        recd = nc.dram_tensor("recd", [97, MT, 512], f16)

        # ---- chunked input DMAs (per-kt so compute can chase the stream) -
        for kt in range(KT):
            nc.sync.dma_start(out=xT[:, kt], in_=xT_d.ap()[:, kt])
            nc.scalar.dma_start(out=wk[:, kt], in_=wk_d.ap()[:, kt])
            nc.gpsimd.dma_start(out=wq[:, kt], in_=wq_d.ap()[:, kt])
        for t, d in [(bq, bq_d), (bk, bk_d), (bv, bv_d), (maskb, maskb_d)]:
            nc.gpsimd.dma_start(out=t[:], in_=d.ap())
        for kt in range(KT):
            nc.scalar.dma_start(out=wv[:, kt], in_=wv_d.ap()[:, kt])
        nc.scalar.dma_start(out=wo[:], in_=wo_d.ap())
        nc.gpsimd.dma_start(out=boT[:], in_=boT_d.ap())

        with tc.tile_pool(name="w1", bufs=1) as w1:
            cos2 = w1.tile([128, S], f32)
            sin2 = w1.tile([128, S], f32)
            prt = w1.tile([128, 128], f16)
            for t, d in [(cos2, cos_d), (sin2, sin_d), (prt, prt_d)]:
                nc.gpsimd.dma_start(out=t[:], in_=d.ap())
            ones_f = w1.tile([128, 128], f32)
            nc.vector.memset(ones_f[:], 1.0)
            nc.vector.tensor_copy(ones[:], ones_f[0:1, :])
            nc.vector.tensor_copy(ones4[:], ones_f[0:97, 0:HEAD_DIM])
            # ones column of V_aug (per head, col 64)
            nc.vector.tensor_copy(
                vv[:, :, :, 64:65].rearrange("p a b c -> p (a b c)"),
                ones_f[:, 0:ST * HSH])
            # only 4 rows per mt are real rowsums; fill the rest so the
            # batched reciprocal never sees uninitialized SBUF
            nc.vector.memset(rssum[:], 1.0)

            def rope_apply(dst, b, c2, ps, pppool, sbpool):
                # row-tile 0 only: RoPE on the first 64 flat channels
                # (rows 64-127 and the hg=1 core get identity via
                # cos=1/sin=0 from the host).
                sl = slice(c2 * 512, (c2 + 1) * 512)
                sinp = sbpool.tile([128, 512], f16, tag="sinp", name="sinp")
                nc.vector.scalar_tensor_tensor(
                    sinp[:], ps[:], b[:, 0:1],
                    sin2[:, sl], op0=OP.add, op1=OP.mult)
                cosp = sbpool.tile([128, 512], f32, tag="cosp", name="cosp")
                nc.vector.scalar_tensor_tensor(
                    cosp[:], ps[:], b[:, 0:1],
                    cos2[:, sl], op0=OP.add, op1=OP.mult)
                pp = pppool.tile([128, 512], f32, tag="pp", name="pp")
                nc.tensor.matmul(out=pp[:], lhsT=prt[:], rhs=sinp[:],
                                 start=True, stop=True)
                nc.vector.tensor_tensor(
                    dst[:, 0, sl], cosp[:], pp[:], op=OP.add)

            def proj_v(st, pool):
                ps = pool.tile([128, DSH], f32, tag="vps", name="vps")
                nc.tensor.matmul(out=ps[:], lhsT=ones[0:1, :],
                                 rhs=bv[:], start=True, stop=False)
                for kt in range(KT):
                    nc.tensor.matmul(
                        out=ps[:],
                        lhsT=xT[:, kt, st * 128:(st + 1) * 128],
                        rhs=wv[:, kt, :],
                        start=False, stop=(kt == KT - 1))
                nc.vector.tensor_copy(
                    vv[:, st, :, 0:64],
                    ps[:].rearrange("p (h d) -> p h d", h=HSH))

            # ---- prologue: K0/Q0 kt-outer (chases the DMA stream), then
            # RoPE chains interleaved with V row-tiles 0-3 so the PE never
            # idles while DVE works through the chains.
            with tc.tile_pool(name="kqps", bufs=1, space="PSUM") as kqps, \
                 tc.tile_pool(name="p1pp", bufs=1, space="PSUM") as p1pp, \
                 tc.tile_pool(name="vps0", bufs=2, space="PSUM") as vps0, \
                 tc.tile_pool(name="p1sb", bufs=3) as p1sb:
                kq = {}
                for di, (dst, w, b) in enumerate(
                        ((kT, wk, bk), (qT, wq, bq))):
                    for c2 in range(2):
                        kq[di, c2] = kqps.tile([128, 512], f32,
                                               tag=f"kq{di}{c2}",
                                               name=f"kq{di}{c2}")
                warm = kqps.tile([128, 512], f32, tag="warm", name="warm")
                for kt in range(KT):
                    for di, (dst, w, b) in enumerate(
                            ((kT, wk, bk), (qT, wq, bq))):
                        for c2 in range(2):
                            sl = slice(c2 * 512, (c2 + 1) * 512)
                            nc.tensor.matmul(
                                out=kq[di, c2][:],
                                lhsT=w[:, kt, 0, :],
                                rhs=xT[:, kt, sl],
                                start=(kt == 0), stop=(kt == KT - 1))
                    if kt == 0:
                        # HAM warm-up: ~4us of back-to-back junk matmuls on
                        # already-landed chunks fills the DMA wait and
                        # un-throttles the PE clock before the real work.
                        for _ in range(10):
                            nc.tensor.matmul(
                                out=warm[:], lhsT=wk[:, 0, 0, :],
                                rhs=xT[:, 0, 0:512], start=True, stop=True)
                # consume the warm tile so nothing can dead-code it away
                nc.vector.tensor_copy(ones_f[0:1, 100:116], warm[0:1, 0:16])
                for i, (di, c2) in enumerate(((0, 0), (1, 0), (0, 1),
                                              (1, 1))):
                    dst, w, b = ((kT, wk, bk), (qT, wq, bq))[di]
                    rope_apply(dst, b, c2, kq[di, c2], p1pp, p1sb)
                    proj_v(i, vps0)

            # ---- phase 2 pools ------------------------------------------
            with tc.tile_pool(name="sch", bufs=2, space="PSUM") as schp, \
                 tc.tile_pool(name="ptp", bufs=8) as ptp, \
                 tc.tile_pool(name="p2sb", bufs=2) as p2sb:

                def emit_scores(mt, c2, kt):
                    qsl = slice(c2 * 512, (c2 + 1) * 512)
                    sc = schp.tile([128, S], f32, tag="sch", name="sch")
                    for hh in range(2):
                        ph = hh * 64
                        nc.tensor.matmul(
                            out=sc[:, hh * 512:hh * 512 + 512],
                            lhsT=kT[ph:ph + 64, mt, kt * 128:(kt + 1) * 128],
                            rhs=qT[ph:ph + 64, mt, qsl],
                            start=True, stop=True, tile_position=(ph, 0))
                    return sc

                def emit_exp(pt, sc, kt):
                    nc.scalar.activation(
                        pt[:], sc[:], AF.Exp,
                        bias=maskb[:, kt:kt + 1], scale=0.125)

                # V row-tiles 4-7 zippered with (mt=0, c2=0) scores+exps:
                # ACT starts its exp marathon while PE streams V.
                pts = {}          # (mt, c2, kt) -> pt tile

                def sc_exp00(kt):
                    sc = emit_scores(0, 0, kt)
                    pt = ptp.tile([128, S], f16, tag="pt", name="pt")
                    pts[0, 0, kt] = pt
                    emit_exp(pt, sc, kt)

                with tc.tile_pool(name="vps2", bufs=2, space="PSUM") as vps:
                    for st in range(4, ST):
                        proj_v(st, vps)
                        sc_exp00(st - 4)
                    for kt in range(4, ST):
                        sc_exp00(kt)

                def proj_gen(mt):
                    # one matmul per next(); bias epilogue on the last.
                    for dst, w, b in ((kT, wk, bk), (qT, wq, bq)):
                        for c2 in range(2):
                            sl = slice(c2 * 512, (c2 + 1) * 512)
                            ps = p1ps.tile([128, 512], f32, tag="ps",
                                           name="ps")
                            for kt in range(KT):
                                nc.tensor.matmul(
                                    out=ps[:], lhsT=w[:, kt, mt, :],
                                    rhs=xT[:, kt, sl],
                                    start=(kt == 0), stop=(kt == KT - 1))
                                if kt == KT - 1:
                                    nc.vector.tensor_scalar(
                                        dst[:, mt, sl], ps[:],
                                        b[:, mt:mt + 1], None, op0=OP.add)
                                yield
                    while True:
                        yield

                def emit_pv(at, mt, c2, kt):
                    pt = pts.pop((mt, c2, kt))
                    for hh in range(2):
                        nc.tensor.matmul(
                            out=at[hh][:],
                            lhsT=vv[:, kt, mt * 2 + hh, :],
                            rhs=pt[:, hh * 512:hh * 512 + 512],
                            start=(kt == 0), stop=(kt == KT - 1))

                def epilogue(at, mt, c2):
                    qsl = slice(c2 * 512, (c2 + 1) * 512)
                    for hh in range(2):
                        r = 32 * (hh * 2 + c2)
                        nc.vector.tensor_copy(
                            rssum[r:r + 1, mt, :], at[hh][64:65, :])
                        nc.vector.tensor_copy(
                            attU[hh * 64:hh * 64 + 64, mt, qsl],
                            at[hh][0:64, :])

                def normalize(mt):
                    # reciprocal of the 4 rowsum rows on DVE; the DRAM-
                    # bounce partition-broadcast and the scaling multiplies
                    # run on GpSimd so the DVE FIFO never waits on DMA.
                    nc.vector.reciprocal_approx_fast(
                        recf[:], rssum[:, mt, :])
                    nc.vector.tensor_copy(recq[:, mt, :], recf[:])
                    nc.gpsimd.dma_start(out=recd.ap()[:, mt, :],
                                        in_=recq[:, mt, :])
                    for hh in range(2):
                        ph = hh * 64
                        rb = p2sb.tile([128, 2, 512], f32, tag="rb",
                                       name="rb")
                        nc.gpsimd.dma_start(
                            out=rb[ph:ph + 64],
                            in_=recd.ap()[64 * hh:64 * hh + 33:32,
                                          mt, :].partition_broadcast(HEAD_DIM))
                        nc.gpsimd.tensor_tensor(
                            attN[ph:ph + 64, mt, :], attU[ph:ph + 64, mt, :],
                            rb[ph:ph + 64].rearrange("p a b -> p (a b)"),
                            op=OP.mult)

                with tc.tile_pool(name="atp", bufs=1, space="PSUM") as atp, \
                     tc.tile_pool(name="p1ps", bufs=2, space="PSUM") as p1ps:
                    def p3partial_gen():
                        # phase-3 partials (mt 0-1 only -- normalized long
                        # ago, so these can never stall the in-order PE
                        # queue): filler for the last row-tile's phases.
                        for dt in range(ST):
                            dsl = slice(dt * 128, (dt + 1) * 128)
                            for c2 in range(2):
                                qsl = slice(c2 * 512, (c2 + 1) * 512)
                                ps = p1ps.tile([128, 512], f32, tag="ps",
                                               name="ps")
                                for mt in range(2):
                                    nc.tensor.matmul(
                                        out=ps[:], lhsT=wo[:, mt, dsl],
                                        rhs=attN[:, mt, qsl],
                                        start=(mt == 0), stop=(mt == 1))
                                    yield
                                nc.vector.tensor_copy(oprt[:, dt, qsl],
                                                      ps[:])
                        while True:
                            yield

                    gen = proj_gen(1)
                    p3gen = p3partial_gen()
                    for mt in range(MT):
                        for c2 in range(2):
                            if c2 == 0 and mt > 0:
                                normalize(mt - 1)
                            at = {hh: atp.tile([65, 512], f32,
                                               tag=f"at{hh}",
                                               name=f"at{hh}")
                                  for hh in range(2)}
                            for kt in range(ST):
                                if not (mt == 0 and c2 == 0):
                                    sc = emit_scores(mt, c2, kt)
                                    pts[mt, c2, kt] = ptp.tile(
                                        [128, S], f16, tag="pt", name="pt")
                                    emit_exp(pts[mt, c2, kt], sc, kt)
                                if mt == 0:
                                    np1, np2 = (1, kt % 2) if c2 == 0 \
                                        else (2, (kt + 1) % 2)
                                else:
                                    np1, np2 = 1, 1
                                for _ in range(np1):
                                    next(gen)
                                # PV lags one kt so it never waits on its
                                # exp inside the in-order PE queue
                                if kt > 0:
                                    emit_pv(at, mt, c2, kt - 1)
                                for _ in range(np2):
                                    next(gen)
                                if mt == 3:
                                    for _ in range(2 if c2 == 0 else 1):
                                        next(p3gen)
                            emit_pv(at, mt, c2, ST - 1)
                            epilogue(at, mt, c2)
                            if c2 == 1:
                                gen = proj_gen(mt + 2) if mt + 2 <= MT - 1 \
                                    else iter(lambda: None, 0)
                    # last row-tile: reciprocal chain on DVE while the PE
                    # drains the remaining phase-3 partials.
                    nc.vector.reciprocal_approx_fast(recf[:],
                                                     rssum[:, 3, :])
                    nc.vector.tensor_copy(recq[:, 3, :], recf[:])
                    for _ in range(10):   # drain partials + final spill
                        next(p3gen)

        # ---- tail: PE-broadcast normalize for mt=3 (no DMA bounce on the
        # critical path) interleaved with the output chains, in fresh deep
        # pools so nothing rotates against the closed phase-2 pools.
        with tc.tile_pool(name="p3t", bufs=6, space="PSUM") as p3t, \
             tc.tile_pool(name="p3s", bufs=4) as p3s:
            for c2 in range(2):
                qsl = slice(c2 * 512, (c2 + 1) * 512)
                for hh in range(2):
                    ph = hh * 64
                    r = 32 * (hh * 2 + c2)
                    rbp = p3t.tile([128, 512], f32, tag="ps", name="rbp")
                    nc.tensor.matmul(
                        out=rbp[0:64, :], lhsT=ones4[r:r + 1, :],
                        rhs=recq[r:r + 1, 3, :],
                        start=True, stop=True, tile_position=(r, 0))
                    nc.vector.tensor_tensor(
                        attN[ph:ph + 64, 3, qsl],
                        attU[ph:ph + 64, 3, qsl],
                        rbp[0:64, :], op=OP.mult)
                # tail: mt=2 + mt=3 on top of the partial, plus bias;
                # DMA out as computed.
                for dt in range(ST):
                    dsl = slice(dt * 128, (dt + 1) * 128)
                    ps = p3t.tile([128, 512], f32, tag="ps", name="ps")
                    nc.tensor.matmul(
                        out=ps[:], lhsT=wo[:, 2, dsl],
                        rhs=attN[:, 2, qsl], start=True, stop=False)
                    nc.tensor.matmul(
                        out=ps[:], lhsT=wo[:, 3, dsl],
                        rhs=attN[:, 3, qsl], start=False, stop=True)
                    ob = p3s.tile([128, 512], f32, tag="ob", name="ob")
                    if dt % 2 == 0:
                        nc.vector.scalar_tensor_tensor(
                            ob[:], ps[:], boT[:, dt:dt + 1],
                            oprt[:, dt, qsl], op0=OP.add, op1=OP.add)
                    else:
                        # ACT is idle post-exps and can read PSUM;
                        # GpSimd finishes the SBUF-only add.
                        obA = p3s.tile([128, 512], f32, tag="obA",
                                       name="obA")
                        nc.scalar.activation(
                            obA[:], ps[:], AF.Identity,
                            bias=boT[:, dt:dt + 1])
                        nc.gpsimd.tensor_tensor(
                            ob[:], obA[:], oprt[:, dt, qsl], op=OP.add)
                    deng = nc.sync if dt % 2 == 0 else nc.scalar
                    deng.dma_start(out=out_d.ap()[dsl, qsl], in_=ob[:])

    nc.compile()
    return nc


def _get_nc():
    if "nc" not in _CACHE:
        _CACHE["nc"] = _build()
    return _CACHE["nc"]


def _prep_inputs(x, mask, freqs, Wq, bq, Wk, bk, Wv, bv, Wo, bo):
    f = np.asarray(freqs, np.float32)[0]              # [S, HEAD_DIM]
    # reference rotates only the first rot_dim=64 channels of the FLAT
    # inner dim -> rows 0-63 of row-tile 0 on the hg=0 core; everything
    # else is identity (cos=1, sin=0).
    cos2 = np.ones((128, S), np.float32)
    sin2 = np.zeros((128, S), np.float32)
    cos2[0:HEAD_DIM] = np.cos(f.T)
    sin2[0:HEAD_DIM] = np.sin(f.T)
    ident = np.ones((128, S), np.float32)
    identz = np.zeros((128, S), np.float32)

    prt = np.zeros((128, 128), np.float16)            # P_rot^T
    i = np.arange(0, 128, 2)
    prt[i + 1, i] = -1.0                              # P_rot[2i, 2i+1] = -1
    prt[i, i + 1] = 1.0                               # P_rot[2i+1, 2i] = +1

    def lhsT_w(w):                                    # [DIM, DSH] -> lhsT tiles
        return np.ascontiguousarray(
            w.reshape(KT, 128, MT, 128).transpose(1, 0, 2, 3)
        ).astype(np.float16)

    def col(b):                                       # [DSH] -> [128, MT]
        return np.ascontiguousarray(b.reshape(MT, 128).T.astype(np.float32))

    in_maps = []
    for b in range(B):
        xT = np.ascontiguousarray(
            np.asarray(x[b], np.float32).T.reshape(KT, 128, S)
            .transpose(1, 0, 2)).astype(np.float16)
        m = np.asarray(mask[b])
        maskb = np.ascontiguousarray(
            np.where(m, 0.0, MASK_NEG).astype(np.float32).reshape(ST, 128).T)
        for hg in range(HG):
            dsl = slice(hg * DSH, (hg + 1) * DSH)
            in_maps.append({
                "xT": xT,
                "wq": lhsT_w(np.asarray(Wq, np.float32)[:, dsl]),
                "wk": lhsT_w(np.asarray(Wk, np.float32)[:, dsl]),
                "wv": np.ascontiguousarray(
                    np.asarray(Wv, np.float32)[:, dsl]
                    .reshape(KT, 128, DSH).transpose(1, 0, 2))
                    .astype(np.float16),
                "wo": np.ascontiguousarray(
                    np.asarray(Wo, np.float32)[dsl, :]
                    .reshape(MT, 128, DIM).transpose(1, 0, 2))
                    .astype(np.float16),
                "bq": col(np.asarray(bq, np.float32)[dsl]),
                "bk": col(np.asarray(bk, np.float32)[dsl]),
                "bv": np.asarray(bv, np.float32)[None, dsl]
                    .astype(np.float16).copy(),
                "boT": np.ascontiguousarray(
                    (np.asarray(bo, np.float32) * 0.5).reshape(ST, 128).T),
                "cos2": cos2 if hg == 0 else ident,
                "sin2": sin2 if hg == 0 else identz,
                "prt": prt,
                "maskb": maskb,
            })
    return in_maps


def run(trace=False, **inputs):
    from concourse import bass_utils
    if trace:
        _install_ntff_hook()
    nc = _get_nc()
    in_maps = _prep_inputs(**inputs)
    res = bass_utils.run_bass_kernel_spmd(
        nc, in_maps, core_ids=list(range(NCORES)), trace=trace)
    mask = np.asarray(inputs["mask"])
    out = np.empty((B, S, DIM), np.float32)
    for b in range(B):
        ot = res.results[2 * b]["out"] + res.results[2 * b + 1]["out"]
        out[b] = ot.T
        out[b][~mask[b]] = 0.0
    return out, res


def kernel(**inputs):
    out, _ = run(trace=False, **inputs)
    return out


def _install_ntff_hook():
    """Register the axon NTFF profiling hook missing from the antenv stub."""
    import sys, types
    try:
        import antenv.axon_hooks  # noqa: F401
        return
    except ImportError:
        pass
    from trn_agent_boot.trn_boot import _ntff_profile_via_ctypes
    hook = _ntff_profile_via_ctypes('/opt/axon/libaxon_pjrt.so')
    mod = types.ModuleType('antenv.axon_hooks')
    mod.get_axon_ntff_profile_hook = lambda: hook
    mod.set_axon_ntff_profile_hook = lambda h: None
    sys.modules['antenv.axon_hooks'] = mod
